# revision 38
# baseline (speedup 1.0000x reference)
"""Trainium2 Bass kernel for MultiHeadSelfAttention with RoPE (bf16 path).

Problem: x[2, 2048, 1024] @ W_qkv[1024, 3072] -> rope(q,k) -> softmax(q k^T/8) v
         -> out @ W_out[1024, 1024].

Sharding (8 cores): batch (2-way) x head-group (4-way, 4 heads each).
Each core computes a partial output [2048, 1024] = attnout_heads @ W_out_rows;
host sums the 4 head-group partials per batch.

All matmuls run in bf16 (inputs pre-cast + DMA-layout-packed on host so every
transfer moves >=2KB lines), accumulating in fp32 PSUM; elementwise work is
bf16 (DVE 2x packed mode).

Schedule: the attention inner loop is paced by ScalarE's exp (~1.2us per
[128,1024] pair-tile vs ~0.86us of PE work per sk tile), so all remaining PE
work is drained INTO those gaps as 1-2 matmul micro-steps per sk:
  qk-proj(pair0) ; v-proj  ->  attention units (ch,pair0) x4 with
  qk-proj(pair1) steps as background  ->  units (ch,pair1) x4 with the
  output projection of finished chunks as background  ->  tail outproj.

On-core dataflow is fully "transposed" so the PE never needs a transpose:
  qT,kT[c, s] = sum_e W[e, c] * xT[e, s]   (lhsT = W slice, rhs = xT)
  rot = Mswap @ qT (PE), q' = qT*cos + rot*sin_signed (DVE)
  scores[sk, sq] per head via K=128 packing: [kT_A|kT_B] against zero-padded
  q ([q_A|0] / [0|q_B]); both heads' 512-col scores land in one [128, 1024]
  PSUM tile so a single ScalarE exp (scale=1/8 folded) serves the pair.
  attnT[sk, sq] -> oT[d, sq] += [v|1]^T attn (ones column gives the softmax
  denominator in row 64 for free); normalize via ones-outer-product broadcast
  + reciprocal + multiply; out_partial[s, e] = att_oT.T @ W_out_rows.
"""

import sys

if "/opt/trn_rl_repo" not in sys.path:
    sys.path.insert(0, "/opt/trn_rl_repo")

import numpy as np

B, S, E = 2, 2048, 1024
ATT = 1024
H = 16
D = 64
HG = 4            # head groups (cores per batch)
HPG = H // HG     # heads per core = 4
PAIRS = HPG // 2  # head pairs per core = 2
ROPE_THETA = 10000.0
N_CORES = 8

CH = 512              # sq chunk for the attention inner loop
N_CH = S // CH        # 4 chunks
N_SK = S // 128       # 16 sk tiles
EK = E // 128         # 8 contraction tiles over embedding dim
NSC = S // 512        # 4 s-chunks for xT staging
WQCOLS = 2 * HPG * D  # 512 qk-weight columns per core

_BUILT = {}


def _build_program():
    import concourse.bacc as bacc
    import concourse.tile as tile
    import concourse.mybir as mybir

    f32 = mybir.dt.float32
    bf16 = mybir.dt.bfloat16
    AF = mybir.ActivationFunctionType

    nc = bacc.Bacc(
        "TRN2",
        target_bir_lowering=False,
        debug=False,
        enable_asserts=False,
        num_devices=N_CORES,
    )

    # Wide operands host-packed to [128, ...] with the 128-row contraction
    # tiles folded into the free dim, so each DMA moves contiguous >=2KB
    # lines per partition. w_qkm = 4 col-blocks of [e, 128] + mswap tail.
    xTs = [
        nc.dram_tensor(f"xT{c}", [128, EK * 512], bf16, kind="ExternalInput").ap()
        for c in range(NSC)
    ]
    w_qkm = nc.dram_tensor(
        "w_qkm", [128, 4 * EK * 128 + 128], bf16, kind="ExternalInput"
    ).ap()
    w_v = nc.dram_tensor("w_v", [128, EK * HPG * D], bf16, kind="ExternalInput").ap()
    w_o = nc.dram_tensor("w_o", [HPG * D, E], bf16, kind="ExternalInput").ap()
    cos_t = nc.dram_tensor("cos_t", [128, S], bf16, kind="ExternalInput").ap()
    sin_t = nc.dram_tensor("sin_t", [128, S], bf16, kind="ExternalInput").ap()
    out = nc.dram_tensor("out", [S, E], f32, kind="ExternalOutput").ap()

    with tile.TileContext(nc) as tc:
        with (
            tc.tile_pool(name="const", bufs=1) as constp,
            tc.tile_pool(name="qkT", bufs=1) as qkTp,
            tc.tile_pool(name="vsb", bufs=1) as vp,
            tc.tile_pool(name="attnout", bufs=1) as aop,
            tc.tile_pool(name="wo", bufs=1) as wop,
            tc.tile_pool(name="xt", bufs=NSC) as xtp,
            tc.tile_pool(name="wqk", bufs=1) as wqkp,
            tc.tile_pool(name="wv", bufs=1) as wvp,
            tc.tile_pool(name="ropes", bufs=3) as ropep,
            tc.tile_pool(name="trig", bufs=1) as trigp,
            tc.tile_pool(name="bgps", bufs=2, space="PSUM") as bgps,
            tc.tile_pool(name="attps", bufs=2, space="PSUM") as attps,
            tc.tile_pool(name="oTps", bufs=1, space="PSUM") as oTps,
            tc.tile_pool(name="expp", bufs=6) as expp,
            tc.tile_pool(name="recipp", bufs=2) as rcp,
            tc.tile_pool(name="osb", bufs=3) as osbp,
        ):
            # onesrow at partition 64 matches the denominator rhs operand
            onesrow = constp.tile([65, 64], bf16, tag="onesrow")
            nc.gpsimd.memset(onesrow[64:65, :], 1.0)
            # ACT warmup: exp table-set load off the critical path
            warm = constp.tile([65, 16], bf16, tag="warm")
            nc.scalar.activation(warm[64:65, :], onesrow[64:65, 0:16], AF.Exp, scale=0.125)
            # PE warmup: dummy matmuls fill the DMA prologue so the HAM
            # clock-gate is already at 8/8 when the first projection lands
            warmw = constp.tile([128, 128], bf16, tag="warmw")
            nc.gpsimd.memset(warmw[:], 0.0)
            warmx = constp.tile([128, 512], bf16, tag="warmx")
            nc.gpsimd.memset(warmx[:], 0.0)
            wps = bgps.tile([128, 512], f32, tag="bg", name="warmps")
            for i in range(28):
                nc.tensor.matmul(
                    wps[:], warmw[:], warmx[:], start=(i == 0), stop=(i == 27)
                )

            qzlo = [qkTp.tile([128, S], bf16, tag=f"qzlo{g}", name=f"qzlo{g}") for g in range(PAIRS)]
            qzhi = [qkTp.tile([128, S], bf16, tag=f"qzhi{g}", name=f"qzhi{g}") for g in range(PAIRS)]
            kT = [qkTp.tile([128, S], bf16, tag=f"kT{g}", name=f"kT{g}") for g in range(PAIRS)]
            for g in range(PAIRS):
                nc.gpsimd.memset(qzlo[g][64:128, :], 0.0)
                nc.gpsimd.memset(qzhi[g][0:64, :], 0.0)
            # v natural + aug ones column, 4 heads: head h occupies cols
            # [65h, 65h+64) = v, col 65h+64 = ones (softmax-denominator row)
            v_c = vp.tile([128, N_SK, 4 * 65], bf16, tag="vc", name="vc")
            for h in range(4):
                nc.gpsimd.memset(v_c[:, :, 65 * h + 64], 1.0)
            att_o = [aop.tile([128, S], bf16, tag=f"ao{g}", name=f"ao{g}") for g in range(PAIRS)]
            wo_sb = [wop.tile([128, E], bf16, tag=f"wo{g}", name=f"wo{g}") for g in range(PAIRS)]

            cos_sb = trigp.tile([128, S], bf16, tag="cos")
            sin_sb = trigp.tile([128, S], bf16, tag="sin")
            # DMA rings: x chunks on sync, weights/trig on scalar.
            # wqkm[:, b, e, :] = W_qk cols [128b, 128b+128) for e-tile e;
            # wqkm[:, 4, 0, :] = mswap (rides the same 2KB-line transfer).
            wqkm = wqkp.tile([128, 4 * EK + 1, 128], bf16, tag="wqk")
            wqk_r = w_qkm.rearrange("p (b c) -> p b c", c=128)
            nc.scalar.dma_start(wqkm[:, 0:EK, :], wqk_r[:, 0:EK, :])
            nc.scalar.dma_start(wqkm[:, EK : 4 * EK + 1, :], wqk_r[:, EK : 4 * EK + 1, :])
            xt_c = []
            for c in range(NSC):
                t = xtp.tile([128, EK, 512], bf16, tag="xt")
                # alternate rings so consecutive chunks stream in parallel
                eng = nc.sync if c % 2 == 0 else nc.scalar
                eng.dma_start(t[:], xTs[c].rearrange("p (e s) -> p e s", e=EK))
                xt_c.append(t)
            nc.sync.dma_start(cos_sb[:], cos_t[:])
            nc.sync.dma_start(sin_sb[:], sin_t[:])
            wv_c = wvp.tile([128, EK, HPG * D], bf16, tag="wv")
            nc.scalar.dma_start(wv_c[:], w_v.rearrange("p (e c) -> p e c", e=EK))
            for g in range(PAIRS):
                nc.scalar.dma_start(wo_sb[g][:], w_o[128 * g : 128 * (g + 1), :])

            def wqk_ap(b, e):
                return wqkm[:, b * EK + e, :]

            msw_sb = wqkm[:, 4 * EK, :]

            # ---------------- micro-step machinery ----------------
            # Background PE work is emitted as single-matmul steps so it
            # drains into the ~0.3us/sk slack of the exp-paced inner loop.
            rope_pend = []

            def rope_tail():
                (g_, dest, sl, raw, ptag, copy_eng) = rope_pend.pop(0)
                rp = (bgps if ptag == "bg" else attps).tile([128, 512], f32, tag=ptag)
                nc.tensor.matmul(rp[:], msw_sb, raw[:], start=True, stop=True)
                rps = ropep.tile([128, 512], bf16, tag="rps")
                if copy_eng == "scalar":
                    nc.scalar.copy(rps[:], rp[:])
                else:
                    nc.vector.tensor_copy(rps[:], rp[:])
                t2 = ropep.tile([128, 512], bf16, tag="t2")
                nc.vector.tensor_mul(t2[:], raw[:], cos_sb[:, sl])
                t1 = ropep.tile([128, 512], bf16, tag="t1")
                nc.vector.tensor_mul(t1[:], rps[:], sin_sb[:, sl])
                if dest is None:
                    nc.vector.tensor_add(qzlo[g_][0:64, sl], t1[0:64, :], t2[0:64, :])
                    nc.vector.tensor_add(qzhi[g_][64:128, sl], t1[64:128, :], t2[64:128, :])
                else:
                    nc.vector.tensor_add(dest[:, sl], t1[:], t2[:])

            def chain_steps(g, ti, c, ptag="bg", copy_eng=None):
                """qk projection chain as EK single-matmul steps. copy_eng
                does the PSUM evacuations: ScalarE in the serial head (idle
                there), VectorE for background chains inside attention (the
                exp stream must own ScalarE)."""
                dest = None if ti == 0 else kT[g]
                b = 2 * ti + g
                sl = slice(512 * c, 512 * (c + 1))
                state = {}

                def mk(e):
                    def step():
                        if e == 0:
                            state["pp"] = (bgps if ptag == "bg" else attps).tile(
                                [128, 512], f32, tag=ptag, name=f"pp{g}{ti}{c}"
                            )
                        nc.tensor.matmul(
                            state["pp"][:],
                            wqk_ap(b, e),
                            xt_c[c][:, e, :],
                            start=(e == 0),
                            stop=(e == EK - 1),
                        )
                        if e == EK - 1:
                            raw = ropep.tile([128, 512], bf16, tag="raw")
                            if copy_eng == "scalar":
                                nc.scalar.copy(raw[:], state["pp"][:])
                            else:
                                nc.vector.tensor_copy(raw[:], state["pp"][:])
                            rope_pend.append((g, dest, sl, raw, ptag, copy_eng))
                            if len(rope_pend) > 1:
                                rope_tail()

                    return step

                return [mk(e) for e in range(EK)]

            def proj_v(st):
                vp_ps = bgps.tile([128, 2 * 128], f32, tag="bg")
                for e in range(EK):
                    nc.tensor.matmul(
                        vp_ps[:],
                        xt_c[st // 4][:, e, 128 * (st % 4) : 128 * (st % 4 + 1)],
                        wv_c[:, e, :],
                        start=(e == 0),
                        stop=(e == EK - 1),
                    )
                nc.vector.tensor_copy(
                    v_c[:, st, 0 : 4 * 65].rearrange("p (h d) -> p h d", h=4)[:, :, 0:64],
                    vp_ps[:].rearrange("p (h d) -> p h d", h=4),
                )

            def outproj_steps(st, tail=False):
                """output projection of one s-tile as two 2-matmul steps
                through the 1-bank bg slots + a DMA step."""
                ssl = slice(128 * st, 128 * (st + 1))
                state = {}

                def half(n):
                    def step():
                        if n == 0:
                            state["ot"] = osbp.tile(
                                [128, E], f32, tag="ot", name=f"ot{st}"
                            )
                        nsl = slice(512 * n, 512 * (n + 1))
                        # tail steps spread across both free PSUM tags so
                        # consecutive halves double-buffer
                        if tail:
                            op = (attps if n == 0 else bgps).tile(
                                [128, 512], f32, tag=("sAB", "bg")[n], name=f"op{st}_{n}"
                            )
                        else:
                            op = bgps.tile([128, 512], f32, tag="bg")
                        for g in range(PAIRS):
                            nc.tensor.matmul(
                                op[:],
                                att_o[g][:, ssl],
                                wo_sb[g][:, nsl],
                                start=(g == 0),
                                stop=(g == PAIRS - 1),
                            )
                        # in the tail ScalarE is idle (exp stream over):
                        # alternate engines; during attention keep ScalarE
                        # exclusively on exps
                        if tail and n == 1:
                            nc.scalar.copy(state["ot"][:, nsl], op[:])
                        else:
                            nc.vector.tensor_copy(state["ot"][:, nsl], op[:])
                        if n == 1:
                            nc.sync.dma_start(out[ssl, :], state["ot"][:])

                    return step

                return [half(0), half(1)]

            def attention_unit(g, ch, bg_queue, bg_budget, prev_norm=None):
                """One (pair, chunk) unit; drains bg_budget steps from
                bg_queue across its 16 exp-paced sk iterations. The
                normalize of the PREVIOUS unit is emitted after this unit's
                first scores+exp so the boundary never stalls the exp
                stream; this unit's own normalize is returned as a
                closure."""
                cslice = slice(CH * ch, CH * (ch + 1))
                hA, hB = 2 * g, 2 * g + 1
                oTA = oTps.tile([65, CH], f32, tag="oTA")
                oTB = oTps.tile([65, CH], f32, tag="oTB")
                exps = []

                def attnv(sk):
                    eAB = exps[sk]
                    nc.tensor.matmul(
                        oTA[:],
                        v_c[:, sk, 65 * hA : 65 * hA + 65],
                        eAB[:, 0:512],
                        start=(sk == 0),
                        stop=(sk == N_SK - 1),
                    )
                    nc.tensor.matmul(
                        oTB[:],
                        v_c[:, sk, 65 * hB : 65 * hB + 65],
                        eAB[:, 512:1024],
                        start=(sk == 0),
                        stop=(sk == N_SK - 1),
                    )

                drained = 0
                for sk in range(N_SK):
                    sksl = slice(128 * sk, 128 * (sk + 1))
                    sAB = attps.tile([128, 1024], f32, tag="sAB")
                    nc.tensor.matmul(
                        sAB[:, 0:512], kT[g][:, sksl], qzlo[g][:, cslice],
                        start=True, stop=True,
                    )
                    nc.tensor.matmul(
                        sAB[:, 512:1024], kT[g][:, sksl], qzhi[g][:, cslice],
                        start=True, stop=True,
                    )
                    eAB = expp.tile([128, 1024], bf16, tag="eAB")
                    nc.scalar.activation(eAB[:], sAB[:], AF.Exp, scale=0.125)
                    exps.append(eAB)
                    if sk == 1 and prev_norm is not None:
                        prev_norm()
                    if sk > 0:
                        attnv(sk - 1)
                    want = (sk + 1) * bg_budget // N_SK
                    while drained < want and bg_queue:
                        bg_queue.pop(0)()
                        drained += 1
                attnv(N_SK - 1)

                # normalize: denominators live in row 64 of oTA/oTB.
                # Stage the denom rows to SBUF, broadcast across 64
                # partitions with a K=1 ones outer-product in the bg slot,
                # reciprocal, one multiply per head (head-interleaved).
                def normalize():
                    oXA = rcp.tile([65, CH], bf16, tag="oX0")
                    oXB = rcp.tile([65, CH], bf16, tag="oX1")
                    nc.vector.tensor_copy(oXA[64:65, :], oTA[64:65, :])
                    nc.vector.tensor_copy(oXB[64:65, :], oTB[64:65, :])
                    dbA = bgps.tile([64, CH], f32, tag="bg")
                    dbB = bgps.tile([64, CH], f32, tag="bg")
                    nc.tensor.matmul(
                        dbA[:], onesrow[64:65, :], oXA[64:65, :], start=True, stop=True
                    )
                    nc.tensor.matmul(
                        dbB[:], onesrow[64:65, :], oXB[64:65, :], start=True, stop=True
                    )
                    rbA = rcp.tile([64, CH], f32, tag="rb0")
                    rbB = rcp.tile([64, CH], f32, tag="rb1")
                    nc.vector.reciprocal_approx_fast(rbA[:], dbA[:])
                    nc.vector.reciprocal_approx_fast(rbB[:], dbB[:])
                    nc.vector.tensor_mul(att_o[g][0:64, cslice], oTA[0:64, :], rbA[:])
                    aoB = rcp.tile([64, CH], bf16, tag="aoB")
                    nc.vector.tensor_mul(aoB[:], oTB[0:64, :], rbB[:])
                    nc.sync.dma_start(att_o[g][64:128, cslice], aoB[:])

                return normalize

            # ---------------- emission ----------------
            # serial head: only what the FIRST attention unit needs — all of
            # pair-0's k (scores read every sk column), the first q chunk,
            # and v. The remaining q chunks ride the background queue just
            # ahead of their units. Chains alternate PSUM slots between the
            # bg tag and the still-idle scores tag.
            for i, (c, ti) in enumerate([(0, 1), (1, 1), (2, 1), (3, 1), (0, 0)]):
                for step in chain_steps(0, ti, c, ptag=("bg", "sAB")[i % 2],
                                        copy_eng="scalar"):
                    step()
            while rope_pend:
                rope_tail()
            for st in range(N_SK):
                proj_v(st)

            # pair-0 attention, with the deferred pair-0 q chunks plus all
            # of pair-1's projection (k first) as background
            g1_steps = []
            for c in range(1, NSC):
                g1_steps.extend(chain_steps(0, 0, c, copy_eng="vector"))
            for ti in (1, 0):
                for c in range(NSC):
                    g1_steps.extend(chain_steps(1, ti, c, copy_eng="vector"))
            pnorm = None
            for ch in range(N_CH):
                pnorm = attention_unit(
                    0, ch, g1_steps,
                    (len(g1_steps) + N_CH - 1 - ch) // (N_CH - ch),
                    prev_norm=pnorm,
                )
            while g1_steps:
                g1_steps.pop(0)()
            while rope_pend:
                rope_tail()

            # pair-1 attention with finished chunks' output projection as
            # background (chunk ch-1 is complete once unit (1, ch-1) done)
            for ch in range(N_CH):
                op_steps = []
                if ch > 0:
                    for st in range(CH * (ch - 1) // 128, CH * ch // 128):
                        op_steps.extend(outproj_steps(st))
                pnorm = attention_unit(1, ch, op_steps, len(op_steps), prev_norm=pnorm)
                while op_steps:
                    op_steps.pop(0)()
            pnorm()
            for st in range(CH * (N_CH - 1) // 128, S // 128):
                for step in outproj_steps(st, tail=True):
                    step()

    nc.compile()
    return nc


def _get_program():
    if "nc" not in _BUILT:
        _BUILT["nc"] = _build_program()
    return _BUILT["nc"]


def _pack_e(a):
    """[E, C] -> [128, EK*C] with row p = concat over e of a[128e+p, :]."""
    Edim, C = a.shape
    return np.ascontiguousarray(
        a.reshape(EK, 128, C).transpose(1, 0, 2).reshape(128, EK * C)
    )


def _host_inputs(x, W_qkv, W_out):
    """Build the 8 per-core input maps (bf16, DMA-packed)."""
    import ml_dtypes

    bf = ml_dtypes.bfloat16
    f = np.float32
    x = np.asarray(x, dtype=f)
    W_qkv = np.asarray(W_qkv, dtype=f)
    W_out = np.asarray(W_out, dtype=f)

    inv_freq = 1.0 / (ROPE_THETA ** (np.arange(0, D, 2, dtype=np.float64) / D))
    p = np.arange(128)
    freq_row = inv_freq[(p % D) // 2]  # [128]
    ang = freq_row[:, None] * np.arange(S, dtype=np.float64)[None, :]  # [128, S]
    cos_t = np.cos(ang).astype(bf)
    sign = np.where(p % 2 == 0, -1.0, 1.0)[:, None]
    sin_t = (np.sin(ang) * sign).astype(bf)

    msw = np.zeros((128, 128), dtype=f)
    msw[p, p ^ 1] = 1.0

    maps = []
    for core in range(N_CORES):
        b, hg = divmod(core, HG)
        hs = [HPG * hg + i for i in range(HPG)]
        w_qk = np.concatenate(
            [W_qkv[:, h * D : (h + 1) * D] for h in hs]
            + [W_qkv[:, ATT + h * D : ATT + (h + 1) * D] for h in hs],
            axis=1,
        )
        w_v = np.concatenate(
            [W_qkv[:, 2 * ATT + h * D : 2 * ATT + (h + 1) * D] for h in hs], axis=1
        )
        w_o = np.concatenate([W_out[h * D : (h + 1) * D, :] for h in hs], axis=0)
        # wqkm: 4 col-blocks of [128, EK*128] + mswap appended
        blocks = [
            _pack_e(np.ascontiguousarray(w_qk[:, 128 * bb : 128 * (bb + 1)]))
            for bb in range(4)
        ]
        w_qkm = np.concatenate(blocks + [msw], axis=1)
        xT = np.ascontiguousarray(x[b].T)
        m = {
            "w_qkm": w_qkm.astype(bf),
            "w_v": _pack_e(w_v).astype(bf),
            "w_o": np.ascontiguousarray(w_o).astype(bf),
            "cos_t": cos_t,
            "sin_t": sin_t,
        }
        for c in range(NSC):
            m[f"xT{c}"] = _pack_e(xT[:, 512 * c : 512 * (c + 1)]).astype(bf)
        maps.append(m)
    return maps


def _gather(res, inputs=None):
    out = np.zeros((B, S, E), dtype=np.float32)
    for core in range(N_CORES):
        b = core // HG
        out[b] += res.results[core]["out"]
    return out


def kernel(x, W_qkv, W_out):
    from concourse.bass_utils import run_bass_kernel_spmd

    nc = _get_program()
    maps = _host_inputs(x, W_qkv, W_out)
    res = run_bass_kernel_spmd(nc, maps, core_ids=list(range(N_CORES)))
    return _gather(res)
